# revision 3
# baseline (speedup 1.0000x reference)
"""Trainium2 Bass kernel for dynamic-scale FP8 GEMM (MixLinear):

    out = (scale_in * scale_w) * (q8(x / scale_in) @ q8(w).T) + bias
    scale_in = max|x| / 448  (global over the whole activation tensor)

Strategy (8 NeuronCores, SPMD, data-parallel over M = B*S = 16384):

  - The per-tensor activation scale is ONE scalar over an input the host
    already holds; it is computed host-side (exact fp16 |max| via a uint16
    view) like the weight-side host prep (quant + packing), and shipped
    pre-broadcast as a [128, 2] f32 input (an on-device gpsimd
    partition_broadcast costs ~7us on the critical path).  No on-device
    amax, no AllGather: cores run fully independently, so the NEFF
    dispatch skew (~30us) no longer rendezvous-stalls every core.
  - Weight is quantized to fp8 e4m3 ON THE HOST (static scale 1.0 -> plain
    RNE cast; |w| << 240 so OCP e4m3fn bits == TRN fp8e4 bits), packed in
    k-PAIR order for the DoubleRow GEMM and grouped NT-MAJOR (4 groups of
    4 n-tiles) so it streams in behind the GEMM's stationary-tile
    progression.
  - TRN fp8_e4m3 saturates at +-240 (vs OCP e4m3fn's +-448), so x is
    quantized with a 2x scale (values land in +-224) and the 2x folds back
    into the dequant scale.
  - x is quantized in NATURAL layout (fp8 [m-part,k]) on DVE
    (~1.3us/block) and transposed on-chip by viewing adjacent fp8 k-PAIRS
    as one fp16 element: a [128m, 1024]-fp16 xbar transpose moves HALF the
    bytes of an fp16 transpose and lands fp8 pairs contiguously.  The
    DoubleRow GEMM reads the pair with a [128, 2(stride 1), m(stride 2)]
    moving AP.  All transposes stay on the Sync HWDGE queue (xbar
    transpose ‖ SBUF-SBUF copy is a HW deadlock; transposes serialized on
    one queue avoid it).
  - The GEMM is ordered in (m-chunk, nt-half) UNITS so the early compute
    only demands half the weight and one x block at a time: four
    single-block lead-in chunks sweep nt 0-7 first (weight groups 0-1),
    then nt 8-15 (groups 2-3), then the wide middle chunks.  The PE array
    is pre-warmed with ~20 dummy fp8 matmuls during the load phase so the
    HAM clock-gate is fully released (8/8) by the first real matmul --
    measured: a dense matmul stream holds 8/8 = 2.4 GHz (109us floor for
    this shape); any idle window drops it to 4/8.
  - PSUM eviction (out = psum*s2 + bias, fp16, output N-major) is split:
    ScalarE activation for nt 0-7, DVE tensor_scalar for nt 8-15, so
    neither engine saturates.  Output DMAs alternate the two HWDGE
    queues.  Deep per-size ev pools absorb the eviction backlog until the
    load phase clears the queues.  Per-core output is [N, M_shard]; the
    host transposes on gather.
"""

import os
import sys

try:
    import concourse  # noqa: F401
except ImportError:  # pragma: no cover
    for _p in ("/opt/trn_rl_repo", "/root/.axon_site/_ro/trn_rl_repo"):
        if os.path.isdir(_p) and _p not in sys.path:
            sys.path.insert(0, _p)

import ml_dtypes
import numpy as np

import concourse.bacc as bacc
import concourse.bass as bass  # noqa: F401
import concourse.mybir as mybir
import concourse.tile as tile
from concourse.bass_utils import run_bass_kernel_spmd

# Problem shapes (hardcoded per contract).
B, S, K, N = 4, 4096, 2048, 2048
M = B * S
N_CORES = 8
MS = M // N_CORES  # 2048 rows of x per core

P = 128
F16 = mybir.dt.float16
F32 = mybir.dt.float32
FP8 = mybir.dt.float8e4

NT_GROUPS = 4   # nt-major weight groups (4 n-tiles = 512 n columns each)
N_WARM = 20     # PE warm-up matmuls (HAM release before the first real mm)

# m-block spans (in 128-row blocks) of the GEMM chunks: four single-block
# lead-in chunks (compute starts after ONE quant+transpose block and only
# ever waits on the next single transpose), wide middle, 256-row tail
# chunks to shorten the end-of-kernel drain.
CHUNK_PLAN = [(0, 1), (1, 2), (2, 3), (3, 4), (4, 8), (8, 12), (12, 14), (14, 16)]
# (chunk, nt-half) execution order: sweep nt 0-7 of the lead-in chunks
# first (weight groups 0-1), then nt 8-15 (groups 2-3 land ~27us).
UNITS = [(0, 0), (1, 0), (2, 0), (3, 0), (0, 1), (1, 1), (2, 1), (3, 1),
         (4, 0), (4, 1), (5, 0), (5, 1), (6, 0), (6, 1), (7, 0), (7, 1)]


def build_nc(ms=MS, k=K, n=N, n_cores=N_CORES):
    """Build + compile the per-core Bass program (SPMD: same NEFF on all cores)."""
    ko = k // P          # k planes (128 each)
    kj = ko // 2         # DoubleRow k steps (256 each)
    mg_n = ms // P       # m blocks (128 rows each)
    nt_tiles = n // P    # GEMM stationary n-tiles
    ntl = nt_tiles // NT_GROUPS  # n-tiles per weight group
    assert k % 256 == 0 and ms % 512 == 0 and n % 256 == 0
    assert CHUNK_PLAN[-1][1] == mg_n

    nc = bacc.Bacc("TRN2", target_bir_lowering=False, debug=False, num_devices=n_cores)
    x = nc.dram_tensor("x", [ms, k], F16, kind="ExternalInput")
    wq8 = nc.dram_tensor("wq8", [P, NT_GROUPS * ko * (n // NT_GROUPS)], FP8,
                         kind="ExternalInput")
    b = nc.dram_tensor("b", [P, n // P], F16, kind="ExternalInput")
    sc = nc.dram_tensor("sc", [P, 2], F32, kind="ExternalInput")
    out_t = nc.dram_tensor("out_t", [n, ms], F16, kind="ExternalOutput")

    with tile.TileContext(nc) as tc:
        with (
            tc.tile_pool(name="big", bufs=1) as big,
            tc.tile_pool(name="small", bufs=1) as small,
            tc.tile_pool(name="ev", bufs=1) as ev,
            tc.tile_pool(name="psum", bufs=2, space="PSUM") as psum,
        ):
            # Persistent SBUF tensors.
            xnat = big.tile([P, mg_n, k], F16)   # x natural: [p, mg, k] = x[mg*128+p, k]
            xqn = big.tile([P, mg_n, k], FP8)    # quantized x, natural layout
            # packed transpose target: fp16 element [q, jj, m] = fp8 pair
            # (k = 2*(jj*128+q) + {0,1}) of column m
            xqT = big.tile([P, kj, ms], F16)
            # w fp8, host packing: [p, g, h, nl] (nt-major groups)
            wq = big.tile([P, NT_GROUPS, ko, n // NT_GROUPS], FP8)

            # ---- Scales + bias (tiny, land first) -----------------------
            sc_bc = small.tile([P, 2], F32)
            nc.sync.dma_start(sc_bc[:], sc.ap())
            inv2s = sc_bc[:, 0:1]   # 224/amax  (quant scale)
            s2 = sc_bc[:, 1:2]      # amax/224  (dequant scale)

            # bias host-prepped as [128, 16] ([p, j] = bias[j*128+p]).
            bias16 = small.tile([P, nt_tiles], F16)
            nc.scalar.dma_start(bias16[:], b.ap())
            bias32 = small.tile([P, nt_tiles], F32)
            nc.vector.tensor_copy(bias32[:], bias16[:])

            # ---- PE warm-up: dummy fp8 DoubleRow matmuls ----------------
            # ~20 x 512-col matmuls keep the PE busy from NEFF start so the
            # HAM clock-gate has released (8/8) before the first real
            # matmul.  Reads a memset scratch tile, accumulates into a
            # dedicated scratch psum bank nobody reads.
            warm_src = small.tile([P, 2, 512], FP8)
            nc.gpsimd.memset(warm_src[:], 0.0)
            warm_ps = psum.tile([P, 512], F32, tag="warm", bufs=1, name="warm_ps")
            for _ in range(N_WARM):
                nc.tensor.matmul(
                    warm_ps[:],
                    lhsT=warm_src[:, :, 0:P],
                    rhs=warm_src[:],
                    start=True,
                    stop=True,
                    perf_mode=mybir.MatmulPerfMode.DoubleRow,
                )

            # ---- Interleaved loads, quant (DVE), packed transposes ------
            xv = x.ap()
            wv = wq8.ap().rearrange("p (g r) -> p g r", g=NT_GROUPS)

            def load_x(eng, b0, nb):
                return eng.dma_start(
                    out=xnat[:, b0:b0 + nb, :],
                    in_=xv[b0 * P:(b0 + nb) * P, :].rearrange(
                        "(b p) k2 -> p b k2", b=nb
                    ),
                )

            def quant_transpose(mg):
                nc.vector.tensor_scalar(
                    xqn[:, mg, :], xnat[:, mg, :], inv2s, None,
                    mybir.AluOpType.mult,
                )
                nc.sync.dma_start(
                    out=xqT[:, :, mg * P:(mg + 1) * P],
                    in_=xqn[:, mg, :].bitcast(F16),
                    transpose=True,
                )

            # Scalar queue: weight groups (needed from GEMM start onward).
            for g in range(NT_GROUPS):
                nc.scalar.dma_start(out=wq[:, g, :, :], in_=wv[:, g, :])
            # Sync queue: x blocks 0-5 as single-block pieces with the
            # transposes threaded in; Scalar queue: x blocks 6-15 behind
            # the weight.
            for mg in range(6):
                load_x(nc.sync, mg, 1)
                quant_transpose(mg)
            for b0 in (6, 8, 10, 12, 14):
                load_x(nc.scalar, b0, 2)
                quant_transpose(b0)
                quant_transpose(b0 + 1)

            # ---- GEMM (fp8 DoubleRow) + fused eviction -------------------
            def rhs_ap(jj, m0, msz):
                return (
                    xqT[:, jj, m0:m0 + msz]
                    .bitcast(FP8)
                    .rearrange("p (m two) -> p two m", two=2)
                )

            def lhsT_ap(jj, nt):
                g, nl0 = divmod(nt, ntl)
                return wq[:, g, 2 * jj:2 * jj + 2, nl0 * P:(nl0 + 1) * P]

            # Per-size ev pools: deep enough that the lead-in chunks never
            # stall on eviction backlog while the load phase still owns
            # both HWDGE queues.  (SBUF: 64x256B + 12x512B + 12x1KB = 34KB.)
            EV_BUFS = {128: 64, 256: 12, 512: 12}
            oi = 0
            for ci, h in UNITS:
                b0, b1 = CHUNK_PLAN[ci]
                m0, msz = b0 * P, (b1 - b0) * P
                for nt in range(h * 8, h * 8 + 8):
                    ps = psum.tile([P, msz], F32, tag="ps", bufs=7,
                                   name=f"ps_{ci}_{nt}")
                    for jj in range(kj):
                        nc.tensor.matmul(
                            ps[:],
                            lhsT=lhsT_ap(jj, nt),
                            rhs=rhs_ap(jj, m0, msz),
                            start=(jj == 0),
                            stop=(jj == kj - 1),
                            perf_mode=mybir.MatmulPerfMode.DoubleRow,
                        )
                    ob = ev.tile([P, msz], F16, tag=f"ob{msz}",
                                 bufs=EV_BUFS[msz], name=f"ob_{ci}_{nt}")
                    if h == 0:
                        nc.scalar.activation(
                            ob[:], ps[:],
                            mybir.ActivationFunctionType.Identity,
                            bias=bias32[:, nt:nt + 1],
                            scale=s2,
                        )
                    else:
                        nc.vector.tensor_scalar(
                            ob[:], ps[:], s2, bias32[:, nt:nt + 1],
                            mybir.AluOpType.mult, mybir.AluOpType.add,
                        )
                    eng = nc.sync if oi % 2 == 0 else nc.scalar
                    eng.dma_start(
                        out_t.ap()[nt * P:(nt + 1) * P, m0:m0 + msz],
                        ob[:],
                    )
                    oi += 1

    nc.compile()
    return nc


_NC_CACHE = {}


def _get_nc():
    if "nc" not in _NC_CACHE:
        _NC_CACHE["nc"] = build_nc()
    return _NC_CACHE["nc"]


def kernel(x, weight, bias):
    x = np.asarray(x, dtype=np.float16).reshape(M, K)
    weight = np.asarray(weight, dtype=np.float16)
    bias = np.asarray(bias, dtype=np.float16)

    nc = _get_nc()

    # Host-side dynamic per-tensor activation scale: exact amax of |x| via
    # the uint16 bit trick (for non-NaN fp16, ordering of (bits & 0x7fff)
    # matches ordering of |value|).  Mirrors the reference's f32 arithmetic:
    # scale_ref = amax/448 (f32 RNE); the TRN fp8e4 grid is driven with 2x
    # that scale (values in +-224 < 240 saturation) and the 2x folds back
    # into the dequant scale s2 = 2*scale_ref (exact).
    amax_bits = (x.view(np.uint16) & np.uint16(0x7FFF)).max()
    amax = np.float32(np.array(amax_bits, dtype=np.uint16).view(np.float16))
    scale_ref = np.maximum(amax / np.float32(448.0), np.float32(1e-12))
    s2 = scale_ref * np.float32(2.0)
    inv2s = np.float32(1.0) / s2
    sc = np.ascontiguousarray(
        np.broadcast_to(np.array([inv2s, s2], dtype=np.float32), (P, 2))
    )

    # Static-weight host prep: quantize (scale 1.0 -> plain RNE cast onto
    # the reference's e4m3fn grid; |w|<240 so bits == TRN fp8e4), transpose
    # to [K, N], and pack rows in k-PAIR order to match the on-chip packed
    # transpose: SBUF wq[q, pi, n] = w8T[k = (pi//2)*256 + 2q + (pi%2), n].
    # Additionally group n NT-MAJOR: [q, g, pi, nl] with n = g*512 + nl, so
    # each weight group is one contiguous 8KB-per-partition DMA.
    w8T = weight.astype(np.float32).astype(ml_dtypes.float8_e4m3fn).T
    wq8 = (
        w8T.reshape(K // 256, 128, 2, N)        # [jj, q, pr, n]
        .transpose(0, 2, 1, 3)                  # [jj, pr, q, n] (pi = 2jj+pr)
        .reshape(K // P, P, N)                  # [pi, q, n]
        .transpose(1, 0, 2)                     # [q, pi, n]
        .reshape(P, K // P, NT_GROUPS, N // NT_GROUPS)  # [q, pi, g, nl]
        .transpose(0, 2, 1, 3)                  # [q, g, pi, nl]
        .reshape(P, K * N // P)
    )
    wq8 = np.ascontiguousarray(wq8)
    bias_pj = np.ascontiguousarray(bias.reshape(N // P, P).T)  # [p, j]
    in_maps = [
        {"x": x[c * MS:(c + 1) * MS], "wq8": wq8, "b": bias_pj, "sc": sc}
        for c in range(N_CORES)
    ]
    trace = bool(int(os.environ.get("KERNEL_TRACE", "0")))
    res = run_bass_kernel_spmd(nc, in_maps, list(range(N_CORES)), trace=trace)
    _NC_CACHE["last_result"] = res

    out = np.empty((M, N), dtype=np.float16)
    for c in range(N_CORES):
        out[c * MS:(c + 1) * MS, :] = res.results[c]["out_t"].T
    return out.reshape(B, S, N)


# revision 10
# speedup vs baseline: 1.2260x; 1.2260x over previous
"""Trainium2 Bass kernel for dynamic-scale FP8 GEMM (MixLinear):

    out = (scale_in * scale_w) * (q8(x / scale_in) @ q8(w).T) + bias
    scale_in = max|x| / 448  (global over the whole activation tensor)

Strategy (8 NeuronCores, SPMD, data-parallel over M = B*S = 16384):

  - The per-tensor activation scale is ONE scalar over an input the host
    already holds; it is computed host-side (exact fp16 |max| via a uint16
    view) like the weight-side host prep (quant + packing), and shipped
    pre-broadcast as a [128, 2] f32 input (an on-device gpsimd
    partition_broadcast costs ~7us on the critical path).  No on-device
    amax, no AllGather: cores run fully independently, so the NEFF
    dispatch skew (~30us) no longer rendezvous-stalls every core.
  - Weight is quantized to fp8 e4m3 ON THE HOST (static scale 1.0 -> plain
    RNE cast; |w| << 240 so OCP e4m3fn bits == TRN fp8e4 bits), packed in
    k-PAIR order for the DoubleRow GEMM and grouped NT-MAJOR (4 groups of
    4 n-tiles) so it streams in behind the GEMM's stationary-tile
    progression.
  - TRN fp8_e4m3 saturates at +-240 (vs OCP e4m3fn's +-448), so x is
    quantized with a 2x scale (values land in +-224) and the 2x folds back
    into the dequant scale.
  - x is quantized in NATURAL layout (fp8 [m-part,k]) on DVE
    (~1.3us/block) and transposed on-chip by viewing adjacent fp8 k-PAIRS
    as one fp16 element: a [128m, 1024]-fp16 xbar transpose moves HALF the
    bytes of an fp16 transpose and lands fp8 pairs contiguously.  The
    DoubleRow GEMM reads the pair with a [128, 2(stride 1), m(stride 2)]
    moving AP.  All transposes stay on the Sync HWDGE queue (xbar
    transpose ‖ SBUF-SBUF copy is a HW deadlock; transposes serialized on
    one queue avoid it).
  - The GEMM is ordered in (m-chunk, nt-half) UNITS so the early compute
    only demands half the weight and one x block at a time: four
    single-block lead-in chunks sweep nt 0-7 first (weight groups 0-1),
    then nt 8-15 (groups 2-3), then the wide middle chunks.  The PE array
    is pre-warmed with ~20 dummy fp8 matmuls during the load phase so the
    HAM clock-gate is fully released (8/8) by the first real matmul --
    measured: a dense matmul stream holds 8/8 = 2.4 GHz (109us floor for
    this shape); any idle window drops it to 4/8.
  - PSUM eviction (out = psum*s2 + bias, fp16, output N-major) runs on
    ScalarE into [128, 4nt, m] half-unit tiles; ONE merged out-DMA per
    half-unit (32 total, Sync queue only) keeps desc-gen off the ScalarE
    stream and away from the Scalar queue's loads.  Deep per-size ev
    pools absorb the eviction backlog until the Sync queue finishes the
    transposes.  Per-core output is [N, M_shard]; the host transposes on
    gather.
"""

import os
import sys

try:
    import concourse  # noqa: F401
except ImportError:  # pragma: no cover
    for _p in ("/opt/trn_rl_repo", "/root/.axon_site/_ro/trn_rl_repo"):
        if os.path.isdir(_p) and _p not in sys.path:
            sys.path.insert(0, _p)

import ml_dtypes
import numpy as np

import concourse.bacc as bacc
import concourse.bass as bass  # noqa: F401
import concourse.mybir as mybir
import concourse.tile as tile
from concourse.bass_utils import run_bass_kernel_spmd

# Problem shapes (hardcoded per contract).
B, S, K, N = 4, 4096, 2048, 2048
M = B * S
N_CORES = 8
MS = M // N_CORES  # 2048 rows of x per core

P = 128
F16 = mybir.dt.float16
F32 = mybir.dt.float32
FP8 = mybir.dt.float8e4

NT_GROUPS = 4   # nt-major weight groups (4 n-tiles = 512 n columns each)
N_WARM = 20     # PE warm-up matmuls (HAM release before the first real mm)

# m-block spans (in 128-row blocks) of the GEMM chunks: four single-block
# lead-in chunks (compute starts after ONE quant+transpose block and only
# ever waits on the next single transpose), wide middle, 256-row tail
# chunks to shorten the end-of-kernel drain.
CHUNK_PLAN = [(0, 1), (1, 2), (2, 3), (3, 4), (4, 8), (8, 12), (12, 14), (14, 16)]
# (chunk, nt-half) execution order: sweep nt 0-7 of the lead-in chunks
# first (weight groups 0-1), then nt 8-15 (groups 2-3 land ~26us).
UNITS = [(0, 0), (1, 0), (2, 0), (3, 0), (0, 1), (1, 1), (2, 1), (3, 1),
         (4, 0), (4, 1), (5, 0), (5, 1), (6, 0), (6, 1), (7, 0), (7, 1)]
# Eviction-pool depth per output width (tiles are [128, 4, msz]: one
# 4-nt half-unit per tile, one out-DMA per tile -> 32 out DMAs total).
EV_BUFS = {128: 10, 256: 3, 512: 3}


def build_nc(ms=MS, k=K, n=N, n_cores=N_CORES):
    """Build + compile the per-core Bass program (SPMD: same NEFF on all cores)."""
    ko = k // P          # k planes (128 each)
    kj = ko // 2         # DoubleRow k steps (256 each)
    mg_n = ms // P       # m blocks (128 rows each)
    nt_tiles = n // P    # GEMM stationary n-tiles
    ntl = nt_tiles // NT_GROUPS  # n-tiles per weight group
    assert k % 256 == 0 and ms % 512 == 0 and n % 256 == 0
    assert CHUNK_PLAN[-1][1] == mg_n

    nc = bacc.Bacc("TRN2", target_bir_lowering=False, debug=False, num_devices=n_cores)
    x = nc.dram_tensor("x", [ms, k], F16, kind="ExternalInput")
    wq8 = nc.dram_tensor("wq8", [P, NT_GROUPS * ko * (n // NT_GROUPS)], FP8,
                         kind="ExternalInput")
    b = nc.dram_tensor("b", [P, n // P], F16, kind="ExternalInput")
    sc = nc.dram_tensor("sc", [P, 2], F32, kind="ExternalInput")
    out_t = nc.dram_tensor("out_t", [n, ms], F16, kind="ExternalOutput")

    with tile.TileContext(nc) as tc:
        with (
            tc.tile_pool(name="big", bufs=1) as big,
            tc.tile_pool(name="small", bufs=1) as small,
            tc.tile_pool(name="ev", bufs=1) as ev,
            tc.tile_pool(name="psum", bufs=2, space="PSUM") as psum,
        ):
            # Persistent SBUF tensors.
            xnat = big.tile([P, mg_n, k], F16)   # x natural: [p, mg, k] = x[mg*128+p, k]
            xqn = big.tile([P, mg_n, k], FP8)    # quantized x, natural layout
            # packed transpose target: fp16 element [q, jj, m] = fp8 pair
            # (k = 2*(jj*128+q) + {0,1}) of column m
            xqT = big.tile([P, kj, ms], F16)
            # w fp8, host packing: [p, g, h, nl] (nt-major groups)
            wq = big.tile([P, NT_GROUPS, ko, n // NT_GROUPS], FP8)

            # ---- Scales + bias (tiny, land first) -----------------------
            sc_bc = small.tile([P, 2], F32)
            nc.sync.dma_start(sc_bc[:], sc.ap())
            inv2s = sc_bc[:, 0:1]   # 224/amax  (quant scale)
            s2 = sc_bc[:, 1:2]      # amax/224  (dequant scale)

            # bias host-prepped as [128, 16] ([p, j] = bias[j*128+p]).
            bias16 = small.tile([P, nt_tiles], F16)
            nc.scalar.dma_start(bias16[:], b.ap())
            bias32 = small.tile([P, nt_tiles], F32)
            nc.vector.tensor_copy(bias32[:], bias16[:])

            # ---- PE warm-up: dummy fp8 DoubleRow matmuls ----------------
            # ~20 x 512-col matmuls keep the PE busy from NEFF start so the
            # HAM clock-gate has released (8/8) before the first real
            # matmul.  Reads a memset scratch tile, accumulates into a
            # dedicated scratch psum bank nobody reads.
            warm_src = small.tile([P, 2, 512], FP8)
            nc.gpsimd.memset(warm_src[:], 0.0)
            # warm_ps shares the "ps" tag so its bank is recycled into the
            # GEMM's psum rotation (all 8 banks) once warm-up ends.
            warm_ps = psum.tile([P, 512], F32, tag="ps", bufs=8, name="warm_ps")
            for _ in range(N_WARM):
                nc.tensor.matmul(
                    warm_ps[:],
                    lhsT=warm_src[:, :, 0:P],
                    rhs=warm_src[:],
                    start=True,
                    stop=True,
                    perf_mode=mybir.MatmulPerfMode.DoubleRow,
                )

            # ---- Interleaved loads, quant (DVE), packed transposes ------
            xv = x.ap()
            wv = wq8.ap().rearrange("p (g r) -> p g r", g=NT_GROUPS)

            def load_x(eng, b0, nb):
                return eng.dma_start(
                    out=xnat[:, b0:b0 + nb, :],
                    in_=xv[b0 * P:(b0 + nb) * P, :].rearrange(
                        "(b p) k2 -> p b k2", b=nb
                    ),
                )

            def transpose_only(mg):
                nc.sync.dma_start(
                    out=xqT[:, :, mg * P:(mg + 1) * P],
                    in_=xqn[:, mg, :].bitcast(F16),
                    transpose=True,
                )

            def quant_transpose(mg):
                nc.vector.tensor_scalar(
                    xqn[:, mg, :], xnat[:, mg, :], inv2s, None,
                    mybir.AluOpType.mult,
                )
                transpose_only(mg)

            # Scalar queue: weight groups first (needed from GEMM start),
            # then x blocks 6-15 streaming behind them.  All desc-gens are
            # emitted up front so nothing on the Scalar engine stream
            # blocks a load.
            for g in range(NT_GROUPS):
                nc.scalar.dma_start(out=wq[:, g, :, :], in_=wv[:, g, :])
            # Sync queue: x blocks 0-5 (2-block lead piece so quant0 lands
            # early) with transposes T0-T5 threaded in FIFO slots that are
            # ready when the queue reaches them (no head-of-line stalls).
            load_x(nc.sync, 0, 2)
            quant_transpose(0)
            quant_transpose(1)
            load_x(nc.sync, 2, 1)
            quant_transpose(2)
            load_x(nc.sync, 3, 1)
            quant_transpose(3)
            load_x(nc.sync, 4, 2)
            quant_transpose(4)
            quant_transpose(5)
            for b0 in (6, 8, 10, 12, 14):
                load_x(nc.scalar, b0, 2)
            for mg in range(6, 16):
                nc.vector.tensor_scalar(
                    xqn[:, mg, :], xnat[:, mg, :], inv2s, None,
                    mybir.AluOpType.mult,
                )
            transpose_only(6)
            transpose_only(7)

            # ---- GEMM (fp8 DoubleRow) + fused eviction -------------------
            def rhs_ap(jj, m0, msz):
                return (
                    xqT[:, jj, m0:m0 + msz]
                    .bitcast(FP8)
                    .rearrange("p (m two) -> p two m", two=2)
                )

            def lhsT_ap(jj, nt):
                g, nl0 = divmod(nt, ntl)
                return wq[:, g, 2 * jj:2 * jj + 2, nl0 * P:(nl0 + 1) * P]

            def gemm_unit(ci, h):
                b0, b1 = CHUNK_PLAN[ci]
                m0, msz = b0 * P, (b1 - b0) * P
                for half in range(2):
                    nt0 = h * 8 + half * 4
                    ob = ev.tile([P, 4, msz], F16, tag=f"ob{msz}",
                                 bufs=EV_BUFS[msz], name=f"ob_{ci}_{nt0}")
                    for i in range(4):
                        nt = nt0 + i
                        ps = psum.tile([P, msz], F32, tag="ps", bufs=8,
                                       name=f"ps_{ci}_{nt}")
                        for jj in range(kj):
                            nc.tensor.matmul(
                                ps[:],
                                lhsT=lhsT_ap(jj, nt),
                                rhs=rhs_ap(jj, m0, msz),
                                start=(jj == 0),
                                stop=(jj == kj - 1),
                                perf_mode=mybir.MatmulPerfMode.DoubleRow,
                            )
                        nc.scalar.activation(
                            ob[:, i, :], ps[:],
                            mybir.ActivationFunctionType.Identity,
                            bias=bias32[:, nt:nt + 1],
                            scale=s2,
                        )
                    # One merged out-DMA per 4-nt half-unit (Sync queue).
                    nc.sync.dma_start(
                        out_t.ap()[nt0 * P:(nt0 + 4) * P, m0:m0 + msz]
                        .rearrange("(i p) m -> p i m", i=4),
                        ob[:],
                    )

            # Lead-in chunks; the remaining transposes are emitted between
            # units so they land on the Sync queue while its out-DMA
            # backlog is still short, well before c5-c7 consume them.
            for u in range(4):
                gemm_unit(*UNITS[u])
            transpose_only(8)
            transpose_only(9)
            for u in range(4, 6):
                gemm_unit(*UNITS[u])
            transpose_only(10)
            transpose_only(11)
            for u in range(6, 8):
                gemm_unit(*UNITS[u])
            transpose_only(12)
            transpose_only(13)
            gemm_unit(*UNITS[8])
            transpose_only(14)
            transpose_only(15)
            for u in range(9, len(UNITS)):
                gemm_unit(*UNITS[u])

    nc.compile()
    return nc


_NC_CACHE = {}


def _get_nc():
    if "nc" not in _NC_CACHE:
        _NC_CACHE["nc"] = build_nc()
    return _NC_CACHE["nc"]


def kernel(x, weight, bias):
    x = np.asarray(x, dtype=np.float16).reshape(M, K)
    weight = np.asarray(weight, dtype=np.float16)
    bias = np.asarray(bias, dtype=np.float16)

    nc = _get_nc()

    # Host-side dynamic per-tensor activation scale: exact amax of |x| via
    # the uint16 bit trick (for non-NaN fp16, ordering of (bits & 0x7fff)
    # matches ordering of |value|).  Mirrors the reference's f32 arithmetic:
    # scale_ref = amax/448 (f32 RNE); the TRN fp8e4 grid is driven with 2x
    # that scale (values in +-224 < 240 saturation) and the 2x folds back
    # into the dequant scale s2 = 2*scale_ref (exact).
    amax_bits = (x.view(np.uint16) & np.uint16(0x7FFF)).max()
    amax = np.float32(np.array(amax_bits, dtype=np.uint16).view(np.float16))
    scale_ref = np.maximum(amax / np.float32(448.0), np.float32(1e-12))
    s2 = scale_ref * np.float32(2.0)
    inv2s = np.float32(1.0) / s2
    sc = np.ascontiguousarray(
        np.broadcast_to(np.array([inv2s, s2], dtype=np.float32), (P, 2))
    )

    # Static-weight host prep: quantize (scale 1.0 -> plain RNE cast onto
    # the reference's e4m3fn grid; |w|<240 so bits == TRN fp8e4), transpose
    # to [K, N], and pack rows in k-PAIR order to match the on-chip packed
    # transpose: SBUF wq[q, pi, n] = w8T[k = (pi//2)*256 + 2q + (pi%2), n].
    # Additionally group n NT-MAJOR: [q, g, pi, nl] with n = g*512 + nl, so
    # each weight group is one contiguous 8KB-per-partition DMA.
    w8T = weight.astype(np.float32).astype(ml_dtypes.float8_e4m3fn).T
    wq8 = (
        w8T.reshape(K // 256, 128, 2, N)        # [jj, q, pr, n]
        .transpose(0, 2, 1, 3)                  # [jj, pr, q, n] (pi = 2jj+pr)
        .reshape(K // P, P, N)                  # [pi, q, n]
        .transpose(1, 0, 2)                     # [q, pi, n]
        .reshape(P, K // P, NT_GROUPS, N // NT_GROUPS)  # [q, pi, g, nl]
        .transpose(0, 2, 1, 3)                  # [q, g, pi, nl]
        .reshape(P, K * N // P)
    )
    wq8 = np.ascontiguousarray(wq8)
    bias_pj = np.ascontiguousarray(bias.reshape(N // P, P).T)  # [p, j]
    in_maps = [
        {"x": x[c * MS:(c + 1) * MS], "wq8": wq8, "b": bias_pj, "sc": sc}
        for c in range(N_CORES)
    ]
    trace = bool(int(os.environ.get("KERNEL_TRACE", "0")))
    res = run_bass_kernel_spmd(nc, in_maps, list(range(N_CORES)), trace=trace)
    _NC_CACHE["last_result"] = res

    out = np.empty((M, N), dtype=np.float16)
    for c in range(N_CORES):
        out[c * MS:(c + 1) * MS, :] = res.results[c]["out_t"].T
    return out.reshape(B, S, N)


# revision 11
# speedup vs baseline: 1.4392x; 1.1740x over previous
"""Trainium2 Bass kernel for dynamic-scale FP8 GEMM (MixLinear):

    out = (scale_in * scale_w) * (q8(x / scale_in) @ q8(w).T) + bias
    scale_in = max|x| / 448  (global over the whole activation tensor)

Strategy (8 NeuronCores, SPMD, data-parallel over M = B*S = 16384):

  - The per-tensor activation scale is ONE scalar over an input the host
    already holds; it is computed host-side (exact fp16 |max| via a uint16
    view) like the weight-side host prep (quant + packing), and shipped
    pre-broadcast as a [128, 2] f32 input.  No on-device amax, no
    AllGather: cores run fully independently, so the NEFF dispatch skew
    no longer rendezvous-stalls every core (the baseline lost ~40us to
    that barrier plus ~30us to the collective+readback chain).
  - Weight is host-quantized to fp8 e4m3 (static scale 1.0 -> plain RNE
    cast; |w| << 240 so OCP e4m3fn bits == TRN fp8e4 bits), packed in
    k-PAIR order for the DoubleRow GEMM and grouped NT-MAJOR (4 groups of
    4 n-tiles) so it streams in behind the GEMM's stationary-tile
    progression.
  - x pieces are loaded ROW-INTERLEAVED ("(p b) k -> p b k"): partition p
    takes `b` consecutive DRAM rows, so each descriptor is b*4KB
    contiguous instead of 4KB -- measured, descriptor size is what sets
    per-queue DMA bandwidth (~150 GB/s at 4KB vs ~270 at 8KB).  The
    resulting m-column permutation is undone on the host during gather.
  - TRN fp8_e4m3 saturates at +-240 (vs OCP e4m3fn's +-448), so x is
    quantized with a 2x scale (values land in +-224) and the 2x folds
    back into the dequant scale.  Quant runs on DVE (~1.3us/block); the
    fp8 natural-layout block is transposed on-chip by viewing adjacent
    fp8 k-PAIRS as one fp16 element (xbar transpose, half the bytes of an
    fp16 transpose, lands fp8 pairs contiguously for the DoubleRow moving
    AP).  All transposes stay on the Sync HWDGE queue (xbar transpose ‖
    SBUF-SBUF copy is a HW deadlock) and are PINNED into their queue
    slots with explicit ordering deps -- the Tile scheduler's DMA model
    is too optimistic and would otherwise run all loads first.
  - The GEMM runs in (m-chunk, nt-quarter) units of 4 stationary tiles:
    the lead-in single-block chunks sweep nt 0-3 / 4-7 / 8-11 / 12-15 in
    an order whose weight demand (1MB per ~6.8us) matches the weight
    group arrival rate, so the first matmul issues at ~18us and the
    stream never starves.  ~22 dummy fp8 matmuls during the load phase
    hold the PE's HAM clock gate open (8/8 = 2.4GHz; any idle window
    drops it to 4/8) so the real stream starts at full rate (109.3us
    floor for 2048^3-per-core fp8 DoubleRow).
  - PSUM eviction (out = psum*s2 + bias, fp16, output N-major) runs on
    ScalarE into [128, 4nt, m] quarter-unit tiles; ONE merged out-DMA per
    quarter (32 total, Sync queue only, after the transposes) keeps
    desc-gen off the Scalar queue's loads and cannot deadlock against
    the xbar.  Per-core output is [N, M_shard]; the host un-permutes and
    transposes on gather.
"""

import os
import sys

try:
    import concourse  # noqa: F401
except ImportError:  # pragma: no cover
    for _p in ("/opt/trn_rl_repo", "/root/.axon_site/_ro/trn_rl_repo"):
        if os.path.isdir(_p) and _p not in sys.path:
            sys.path.insert(0, _p)

import ml_dtypes
import numpy as np

import concourse.bacc as bacc
import concourse.bass as bass  # noqa: F401
import concourse.mybir as mybir
import concourse.tile as tile
from concourse.bass_utils import run_bass_kernel_spmd

# Problem shapes (hardcoded per contract).
B, S, K, N = 4, 4096, 2048, 2048
M = B * S
N_CORES = 8
MS = M // N_CORES  # 2048 rows of x per core

P = 128
F16 = mybir.dt.float16
F32 = mybir.dt.float32
FP8 = mybir.dt.float8e4

NT_GROUPS = 4   # nt-major weight groups (4 n-tiles = 512 n columns each)
N_WARM = 22     # PE warm-up matmuls (HAM release before the first real mm)

# x load pieces as (first block, n blocks): Sync queue carries blocks 0-9
# (lead blocks small for the earliest GEMM start, blocks 4-7 as one
# 16KB-descriptor piece), Scalar queue carries w then blocks 10-15.
PIECES_SYNC = [(0, 2), (2, 1), (3, 1), (4, 4), (8, 2)]
PIECES_SCALAR = [(10, 6)]

# m-block spans (in 128-row blocks) of the GEMM chunks.
CHUNK_PLAN = [(0, 1), (1, 2), (2, 3), (3, 4), (4, 8), (8, 12), (12, 14), (14, 16)]
# (chunk, nt-quarter) execution order: the lead-in sweeps nt quarters in
# an order whose weight-group demand matches the 1MB-per-5.6us arrival
# rate (q0 pass -> q1 pass -> q2/q3), while m-blocks 0-3 are consumed in
# load order.
UNITS = [(0, 0), (1, 0), (0, 1), (1, 1), (2, 0), (3, 0), (2, 1), (3, 1),
         (0, 2), (1, 2), (0, 3), (1, 3), (2, 2), (3, 2), (2, 3), (3, 3),
         (4, 0), (4, 1), (4, 2), (4, 3), (5, 0), (5, 1), (5, 2), (5, 3),
         (6, 0), (6, 1), (6, 2), (6, 3), (7, 0), (7, 1), (7, 2), (7, 3)]
# Eviction-pool depth per output width (tiles are [128, 4, msz]: one
# 4-nt quarter-unit per tile, one merged out-DMA per tile).
EV_BUFS = {128: 16, 256: 2, 512: 4}


def build_nc(ms=MS, k=K, n=N, n_cores=N_CORES):
    """Build + compile the per-core Bass program (SPMD: same NEFF on all cores)."""
    ko = k // P          # k planes (128 each)
    kj = ko // 2         # DoubleRow k steps (256 each)
    mg_n = ms // P       # m blocks (128 rows each)
    nt_tiles = n // P    # GEMM stationary n-tiles
    ntl = nt_tiles // NT_GROUPS  # n-tiles per weight group
    assert k % 256 == 0 and ms % 512 == 0 and n % 256 == 0
    assert CHUNK_PLAN[-1][1] == mg_n

    nc = bacc.Bacc("TRN2", target_bir_lowering=False, debug=False, num_devices=n_cores)
    x = nc.dram_tensor("x", [ms, k], F16, kind="ExternalInput")
    wq8 = nc.dram_tensor("wq8", [P, NT_GROUPS * ko * (n // NT_GROUPS)], FP8,
                         kind="ExternalInput")
    b = nc.dram_tensor("b", [P, n // P], F16, kind="ExternalInput")
    sc = nc.dram_tensor("sc", [P, 2], F32, kind="ExternalInput")
    out_t = nc.dram_tensor("out_t", [n, ms], F16, kind="ExternalOutput")

    with tile.TileContext(nc) as tc:
        with (
            tc.tile_pool(name="big", bufs=1) as big,
            tc.tile_pool(name="small", bufs=1) as small,
            tc.tile_pool(name="ev", bufs=1) as ev,
            tc.tile_pool(name="psum", bufs=2, space="PSUM") as psum,
        ):
            # Persistent SBUF tensors.
            xnat = big.tile([P, mg_n, k], F16)   # x natural (row-interleaved blocks)
            xqn = big.tile([P, mg_n, k], FP8)    # quantized x, natural layout
            # packed transpose target: fp16 element [q, jj, m] = fp8 pair
            # (k = 2*(jj*128+q) + {0,1}) of column m
            xqT = big.tile([P, kj, ms], F16)
            # w fp8, host packing: [p, g, h, nl] (nt-major groups)
            wq = big.tile([P, NT_GROUPS, ko, n // NT_GROUPS], FP8)

            # ---- Scales + bias (tiny, land first) -----------------------
            sc_bc = small.tile([P, 2], F32)
            nc.sync.dma_start(sc_bc[:], sc.ap())
            inv2s = sc_bc[:, 0:1]   # 224/amax  (quant scale)
            s2 = sc_bc[:, 1:2]      # amax/224  (dequant scale)

            bias16 = small.tile([P, nt_tiles], F16)
            nc.scalar.dma_start(bias16[:], b.ap())
            bias32 = small.tile([P, nt_tiles], F32)
            nc.vector.tensor_copy(bias32[:], bias16[:])

            # ---- PE warm-up: dummy fp8 DoubleRow matmuls ----------------
            warm_src = small.tile([P, 2, 512], FP8)
            nc.gpsimd.memset(warm_src[:], 0.0)
            # warm_ps shares the "ps" tag so its bank is recycled into the
            # GEMM's psum rotation (all 8 banks) once warm-up ends.
            warm_ps = psum.tile([P, 512], F32, tag="ps", bufs=8, name="warm_ps")
            for _ in range(N_WARM):
                nc.tensor.matmul(
                    warm_ps[:],
                    lhsT=warm_src[:, :, 0:P],
                    rhs=warm_src[:],
                    start=True,
                    stop=True,
                    perf_mode=mybir.MatmulPerfMode.DoubleRow,
                )

            # ---- Loads, quant (DVE), packed transposes ------------------
            xv = x.ap()
            wv = wq8.ap().rearrange("p (g r) -> p g r", g=NT_GROUPS)

            def load_x(eng, b0, nb):
                # Row-interleaved: partition p <- rows b0*128 + nb*p + j,
                # one nb*4KB contiguous descriptor per partition.
                return eng.dma_start(
                    out=xnat[:, b0:b0 + nb, :],
                    in_=xv[b0 * P:(b0 + nb) * P, :].rearrange(
                        "(p b) k2 -> p b k2", b=nb
                    ),
                )

            def quant(mg):
                nc.vector.tensor_scalar(
                    xqn[:, mg, :], xnat[:, mg, :], inv2s, None,
                    mybir.AluOpType.mult,
                )

            def transpose_only(mg):
                return nc.sync.dma_start(
                    out=xqT[:, :, mg * P:(mg + 1) * P],
                    in_=xqn[:, mg, :].bitcast(F16),
                    transpose=True,
                )

            # Scalar queue: weight groups, then x blocks 10-15.
            for g in range(NT_GROUPS):
                nc.scalar.dma_start(out=wq[:, g, :, :], in_=wv[:, g, :])
            for b0, nb in PIECES_SCALAR:
                load_x(nc.scalar, b0, nb)
            # Sync queue: x pieces with transposes PINNED between them
            # (explicit ordering deps: the scheduler's DMA model is too
            # optimistic and would otherwise push every transpose behind
            # all the loads, starving the GEMM).
            li = load_x(nc.sync, 0, 2)
            quant(0)
            t0i = transpose_only(0)
            quant(1)
            t1i = transpose_only(1)
            li = load_x(nc.sync, 2, 1)
            tile.add_dep_helper(li.ins, t1i.ins, sync=False,
                                reason="pin T0/T1 before x2 on the Sync queue")
            quant(2)
            t2i = transpose_only(2)
            li = load_x(nc.sync, 3, 1)
            tile.add_dep_helper(li.ins, t2i.ins, sync=False,
                                reason="pin T2 before x3 on the Sync queue")
            quant(3)
            t3i = transpose_only(3)
            li = load_x(nc.sync, 4, 4)
            tile.add_dep_helper(li.ins, t3i.ins, sync=False,
                                reason="pin T3 before x4-7 on the Sync queue")
            for mg in range(4, 8):
                quant(mg)
            t7i = None
            for mg in range(4, 8):
                t7i = transpose_only(mg)
            li = load_x(nc.sync, 8, 2)
            tile.add_dep_helper(li.ins, t7i.ins, sync=False,
                                reason="pin T4-7 before x8-9 on the Sync queue")
            for mg in range(8, 10):
                quant(mg)
            for mg in range(8, 10):
                transpose_only(mg)
            for mg in range(10, 16):
                quant(mg)

            # ---- GEMM (fp8 DoubleRow) + fused eviction -------------------
            def rhs_ap(jj, m0, msz):
                return (
                    xqT[:, jj, m0:m0 + msz]
                    .bitcast(FP8)
                    .rearrange("p (m two) -> p two m", two=2)
                )

            def lhsT_ap(jj, nt):
                g, nl0 = divmod(nt, ntl)
                return wq[:, g, 2 * jj:2 * jj + 2, nl0 * P:(nl0 + 1) * P]

            def gemm_quarter(ci, q):
                b0, b1 = CHUNK_PLAN[ci]
                m0, msz = b0 * P, (b1 - b0) * P
                nt0 = q * 4
                ob = ev.tile([P, 4, msz], F16, tag=f"ob{msz}",
                             bufs=EV_BUFS[msz], name=f"ob_{ci}_{nt0}")
                for i in range(4):
                    nt = nt0 + i
                    ps = psum.tile([P, msz], F32, tag="ps", bufs=8,
                                   name=f"ps_{ci}_{nt}")
                    for jj in range(kj):
                        nc.tensor.matmul(
                            ps[:],
                            lhsT=lhsT_ap(jj, nt),
                            rhs=rhs_ap(jj, m0, msz),
                            start=(jj == 0),
                            stop=(jj == kj - 1),
                            perf_mode=mybir.MatmulPerfMode.DoubleRow,
                        )
                    nc.scalar.activation(
                        ob[:, i, :], ps[:],
                        mybir.ActivationFunctionType.Identity,
                        bias=bias32[:, nt:nt + 1],
                        scale=s2,
                    )
                # One merged out-DMA per 4-nt quarter-unit (Sync queue).
                nc.sync.dma_start(
                    out_t.ap()[nt0 * P:(nt0 + 4) * P, m0:m0 + msz]
                    .rearrange("(i p) m -> p i m", i=4),
                    ob[:],
                )

            # Lead-in units; transposes T10-15 are emitted between units
            # so they take Sync-queue slots ahead of most of the output
            # backlog, well before chunks 5-7 consume them.
            for u in range(4):
                gemm_quarter(*UNITS[u])
            transpose_only(10)
            transpose_only(11)
            for u in range(4, 12):
                gemm_quarter(*UNITS[u])
            transpose_only(12)
            transpose_only(13)
            for u in range(12, 16):
                gemm_quarter(*UNITS[u])
            transpose_only(14)
            transpose_only(15)
            for u in range(16, len(UNITS)):
                gemm_quarter(*UNITS[u])

    nc.compile()
    return nc


_NC_CACHE = {}


def _get_nc():
    if "nc" not in _NC_CACHE:
        _NC_CACHE["nc"] = build_nc()
    return _NC_CACHE["nc"]


def _col_of_row():
    """out_t column index for each x row (inverse of the row-interleaved
    load permutation): piece (b0, nb) puts x row b0*128 + nb*p + j into
    logical block b0+j at column position p."""
    col = np.empty(MS, dtype=np.int64)
    for b0, nb in PIECES_SYNC + PIECES_SCALAR:
        off = np.arange(nb * P)
        col[b0 * P + off] = (b0 + off % nb) * P + off // nb
    return col


def kernel(x, weight, bias):
    x = np.asarray(x, dtype=np.float16).reshape(M, K)
    weight = np.asarray(weight, dtype=np.float16)
    bias = np.asarray(bias, dtype=np.float16)

    nc = _get_nc()

    # Host-side dynamic per-tensor activation scale: exact amax of |x| via
    # the uint16 bit trick (for non-NaN fp16, ordering of (bits & 0x7fff)
    # matches ordering of |value|).  Mirrors the reference's f32
    # arithmetic: scale_ref = amax/448 (f32 RNE); the TRN fp8e4 grid is
    # driven with 2x that scale (values in +-224 < 240 saturation) and the
    # 2x folds back into the dequant scale s2 = 2*scale_ref (exact).
    amax_bits = (x.view(np.uint16) & np.uint16(0x7FFF)).max()
    amax = np.float32(np.array(amax_bits, dtype=np.uint16).view(np.float16))
    scale_ref = np.maximum(amax / np.float32(448.0), np.float32(1e-12))
    s2 = scale_ref * np.float32(2.0)
    inv2s = np.float32(1.0) / s2
    sc = np.ascontiguousarray(
        np.broadcast_to(np.array([inv2s, s2], dtype=np.float32), (P, 2))
    )

    # Static-weight host prep: quantize (scale 1.0 -> plain RNE cast onto
    # the reference's e4m3fn grid; |w|<240 so bits == TRN fp8e4), transpose
    # to [K, N], and pack rows in k-PAIR order to match the on-chip packed
    # transpose: SBUF wq[q, pi, n] = w8T[k = (pi//2)*256 + 2q + (pi%2), n].
    # Additionally group n NT-MAJOR: [q, g, pi, nl] with n = g*512 + nl, so
    # each weight group is one contiguous 8KB-per-partition DMA.
    w8T = weight.astype(np.float32).astype(ml_dtypes.float8_e4m3fn).T
    wq8 = (
        w8T.reshape(K // 256, 128, 2, N)        # [jj, q, pr, n]
        .transpose(0, 2, 1, 3)                  # [jj, pr, q, n] (pi = 2jj+pr)
        .reshape(K // P, P, N)                  # [pi, q, n]
        .transpose(1, 0, 2)                     # [q, pi, n]
        .reshape(P, K // P, NT_GROUPS, N // NT_GROUPS)  # [q, pi, g, nl]
        .transpose(0, 2, 1, 3)                  # [q, g, pi, nl]
        .reshape(P, K * N // P)
    )
    wq8 = np.ascontiguousarray(wq8)
    bias_pj = np.ascontiguousarray(bias.reshape(N // P, P).T)  # [p, j]
    in_maps = [
        {"x": x[c * MS:(c + 1) * MS], "wq8": wq8, "b": bias_pj, "sc": sc}
        for c in range(N_CORES)
    ]
    trace = bool(int(os.environ.get("KERNEL_TRACE", "0")))
    res = run_bass_kernel_spmd(nc, in_maps, list(range(N_CORES)), trace=trace)
    _NC_CACHE["last_result"] = res

    col = _col_of_row()
    out = np.empty((M, N), dtype=np.float16)
    for c in range(N_CORES):
        out[c * MS:(c + 1) * MS, :] = res.results[c]["out_t"][:, col].T
    return out.reshape(B, S, N)
